# revision 4
# baseline (speedup 1.0000x reference)
"""Trainium2 Bass kernel for nn_DistLoss (retrieval_knn, brute-force chamfer-style loss).

reference computes: sum over M targets of the squared distance to the nearest
of S*N surface points.

Strategy (8 NeuronCores, SPMD, targets sharded along M):
  dist[m, j] = ||t_m||^2 + ||s_j||^2 - 2 t_m . s_j
  min over j is shift-invariant in ||t_m||^2, so compute
  p[m, j] = sum_k (s_jk^2 - 2 t_mk s_jk) with a single K=6 matmul:
     lhsT rows = (-2 tx, -2 ty, -2 tz, 1, 1, 1)   [6, 128]  (stationary)
     rhs  rows = ( sx,    sy,    sz, sx^2, sy^2, sz^2) [6, 512] (moving)
  -> PSUM [128 targets, 512 points].  reduce_min over the free axis on DVE,
  add ||t_m||^2 back per target, then sum everything.
"""

import sys

sys.path.insert(0, "/opt/trn_rl_repo")

import numpy as np

# Problem shape (hardcoded per contract)
S, N, K = 4, 4096, 3
M = 16384
SN = S * N  # 16384
N_CORES = 8
M_SHARD = M // N_CORES  # 2048
MT = M_SHARD // 128  # 16 target tiles per core
CHUNK = 512  # matmul moving free dim (one PSUM bank of fp32)
GROUP = 4  # chunks per PSUM tile (4 banks)
N_CHUNKS = SN // CHUNK  # 32
N_GROUPS = N_CHUNKS // GROUP  # 8
SQ = 2048  # surface-square chunk (overlap ACT with main loop)

_CACHE = {}


def _build():
    if "nc" in _CACHE:
        return _CACHE["nc"]

    from contextlib import ExitStack

    import concourse.bass as bass  # noqa: F401
    import concourse.tile as tile
    from concourse import bacc, mybir

    f32 = mybir.dt.float32
    nc = bacc.Bacc(
        "TRN2", target_bir_lowering=False, debug=False, num_devices=N_CORES
    )

    surf_t = nc.dram_tensor("surf_t", [3, SN], f32, kind="ExternalInput").ap()
    tgt_t = nc.dram_tensor("tgt_t", [3, M_SHARD], f32, kind="ExternalInput").ap()
    tgt_n = nc.dram_tensor("tgt_n", [128, MT * 3], f32, kind="ExternalInput").ap()
    ones_row = nc.dram_tensor(
        "ones_row", [1, M_SHARD], f32, kind="ExternalInput"
    ).ap()
    out = nc.dram_tensor("out", [1, 1], f32, kind="ExternalOutput").ap()

    with tile.TileContext(nc) as tc, ExitStack() as ctx:
        sing = ctx.enter_context(tc.tile_pool(name="sing", bufs=1))
        psum = ctx.enter_context(
            tc.tile_pool(name="psum", bufs=2, space="PSUM")
        )

        # --- surfaces: [6, SN] = coords on partitions 0-2, squares on 3-5.
        # Compute ops must start at an aligned partition base, so square in a
        # base-0 staging tile and DMA the result into partitions 3-5.
        surf_aug = sing.tile([6, SN], f32)
        sq_stage = sing.tile([3, SN], f32)
        nc.sync.dma_start(surf_aug[0:3, :], surf_t[:])
        nc.sync.dma_start(sq_stage[:], surf_t[:])
        for c in range(SN // SQ):
            sl = sq_stage[0:3, c * SQ : (c + 1) * SQ]
            nc.scalar.activation(sl, sl, mybir.ActivationFunctionType.Square)
            nc.sync.dma_start(surf_aug[3:6, c * SQ : (c + 1) * SQ], sl)

        # --- targets: [6, M_SHARD] = -2*coords on 0-2, ones on 3-5
        tgt_aug = sing.tile([6, M_SHARD], f32)
        nc.sync.dma_start(tgt_aug[0:3, :], tgt_t[:])
        nc.scalar.mul(tgt_aug[0:3, :], tgt_aug[0:3, :], -2.0)
        for r in range(3, 6):
            nc.sync.dma_start(tgt_aug[r : r + 1, :], ones_row[:])

        # --- per-target ||t||^2 in the [128 part, MT] output layout
        b2src = sing.tile([128, MT * 3], f32)
        nc.sync.dma_start(b2src[:], tgt_n[:])
        nc.scalar.activation(
            b2src[:], b2src[:], mybir.ActivationFunctionType.Square
        )
        b2grid = sing.tile([128, MT], f32)
        nc.vector.tensor_reduce(
            b2grid[:],
            b2src[:].rearrange("p (i k) -> p i k", k=3),
            axis=mybir.AxisListType.X,
            op=mybir.AluOpType.add,
        )

        # --- main loop: for each target tile, sweep all surface chunks
        allmins = sing.tile([128, MT * N_GROUPS], f32)
        for i in range(MT):
            lhsT = tgt_aug[0:6, i * 128 : (i + 1) * 128]
            for g in range(N_GROUPS):
                pt = psum.tile([128, GROUP * CHUNK], f32, tag="pt")
                for jj in range(GROUP):
                    j = g * GROUP + jj
                    nc.tensor.matmul(
                        pt[:, jj * CHUNK : (jj + 1) * CHUNK],
                        lhsT,
                        surf_aug[0:6, j * CHUNK : (j + 1) * CHUNK],
                    )
                col = i * N_GROUPS + g
                nc.vector.tensor_reduce(
                    allmins[:, col : col + 1],
                    pt[:],
                    axis=mybir.AxisListType.X,
                    op=mybir.AluOpType.min,
                )

        # --- finish: min over groups, add b2, sum over all targets
        redm = sing.tile([128, MT], f32)
        nc.vector.tensor_reduce(
            redm[:],
            allmins[:].rearrange("p (i g) -> p i g", g=N_GROUPS),
            axis=mybir.AxisListType.X,
            op=mybir.AluOpType.min,
        )
        dists = sing.tile([128, MT], f32)
        nc.vector.tensor_add(dists[:], redm[:], b2grid[:])
        colsum = sing.tile([128, 1], f32)
        nc.vector.tensor_reduce(
            colsum[:], dists[:], axis=mybir.AxisListType.X, op=mybir.AluOpType.add
        )
        ones = sing.tile([128, 1], f32)
        nc.any.memset(ones[:], 1.0)
        fin = psum.tile([128, GROUP * CHUNK], f32, tag="pt")
        nc.tensor.matmul(fin[:1, :1], colsum[:], ones[:])
        res = sing.tile([1, 1], f32)
        nc.scalar.copy(res[:], fin[:1, :1])
        nc.sync.dma_start(out[:], res[:])

    nc.compile()
    _CACHE["nc"] = nc
    return nc


def _run(inputs, trace=False):
    from concourse.bass_utils import run_bass_kernel_spmd

    surfaces = np.asarray(inputs["surfaces"], dtype=np.float32)
    targets = np.asarray(inputs["targets"], dtype=np.float32)
    assert surfaces.shape == (S, N, K)
    assert targets.shape == (M, K)

    nc = _build()

    surf_flat = surfaces.reshape(SN, 3)
    surf_t = np.ascontiguousarray(surf_flat.T)  # [3, SN]
    in_maps = []
    for c in range(N_CORES):
        shard = targets[c * M_SHARD : (c + 1) * M_SHARD]  # [2048, 3]
        tgt_t = np.ascontiguousarray(shard.T)  # [3, 2048]
        tgt_n = np.ascontiguousarray(
            shard.reshape(MT, 128, 3).transpose(1, 0, 2).reshape(128, MT * 3)
        )
        in_maps.append(
            {
                "surf_t": surf_t,
                "tgt_t": tgt_t,
                "tgt_n": tgt_n,
                "ones_row": np.ones((1, M_SHARD), dtype=np.float32),
            }
        )

    bkr = run_bass_kernel_spmd(
        nc, in_maps, list(range(N_CORES)), trace=trace
    )
    partials = np.array(
        [bkr.results[c]["out"][0, 0] for c in range(N_CORES)], dtype=np.float32
    )
    total = np.float32(partials.sum(dtype=np.float32))
    return np.asarray(total, dtype=np.float32), bkr


def kernel(surfaces, targets):
    out, _ = _run({"surfaces": surfaces, "targets": targets}, trace=False)
    return out
